# revision 45
# baseline (speedup 1.0000x reference)
"""Causal multi-head self-attention (B=4, S=2048, D=1024, H=16) on 8 TRN2
NeuronCores.

Sharding: core c = (batch b = c//2, head-half = c%2). Each core computes, for
its batch and its 8 heads: QKV projections (+RoPE), causal softmax attention,
and a row-sharded output projection. The host sums the two partial y's per
batch.

v5 design:
  - fp16 everywhere on-chip except PSUM (fp32) and the y output; data ranges
    are small (|score| < 5, den < 4e3). fp16 gives 1 cyc/row matmuls at any
    moving width, 2x DVE tensor_tensor, and half the DMA/SBUF footprint.
  - Fully woven schedule: pair hc+1's q/k projections (and, on pair 0, the
    v projection slices) are interleaved between the attention j-blocks, so
    the PE never drains and the HAM clock stays at 2.4 GHz; ScalarE's exp
    stream (the 2nd-busiest engine) overlaps projection matmuls.
  - RoPE: DVE copies the projection PSUM to fp16 SBUF; the +-32 partition
    swap runs on the PE as a constant permutation matmul; DVE does one mul
    by the pre-swapped signed sin table and the final add (2x fp16 rate).
  - Scores are packed causally: head0 at cols [w0:512] and head1 at
    [512:1024-w0] of one PSUM pair-tile, so each step's exp is a single
    contiguous ScalarE instruction covering exactly the causal range.
  - Causal masking of the diagonal 128-block via two tiny fp16 tri-mask
    multiplies on DVE (2x rate); v carries a 65th ones-column so the AV
    matmul emits the softmax denominator as PSUM row 64.
  - Denominators: batched per pair -> DRAM bounce -> fast-approx reciprocal
    -> partition-broadcast DMA -> one fp16 multiply per (pair, j).
"""

import numpy as np

B, S, D = 4, 2048, 1024
NUM_HEADS = 16
THETA = 10000.0
DH = 64
N_CORES = 8
P = 128

_CACHE = {}


def build_nc():
    """Build the single-core SPMD Bass program (identical on all 8 cores)."""
    import concourse.mybir as mybir
    import concourse.tile as tile
    from concourse import bacc
    from concourse.bass import ts

    F16 = mybir.dt.float16
    F32 = mybir.dt.float32
    Act = mybir.ActivationFunctionType

    nc = bacc.Bacc(trn_type="TRN2")
    xT_d = nc.dram_tensor("xT", [D, S], F16, kind="ExternalInput")
    wqT_d = nc.dram_tensor("wqT", [D, 512], F16, kind="ExternalInput")
    wkT_d = nc.dram_tensor("wkT", [D, 512], F16, kind="ExternalInput")
    wvT_d = nc.dram_tensor("wvT", [D, 512], F16, kind="ExternalInput")
    woT_d = nc.dram_tensor("woT", [512, D], F16, kind="ExternalInput")
    cosT_d = nc.dram_tensor("cosT", [P, S], F16, kind="ExternalInput")
    sinT_d = nc.dram_tensor("sinT", [P, S], F16, kind="ExternalInput")
    tri_d = nc.dram_tensor("tri", [P, P], F16, kind="ExternalInput")
    rswp_d = nc.dram_tensor("rswp", [P, P], F16, kind="ExternalInput")
    y_d = nc.dram_tensor("y", [S, D], F32, kind="ExternalOutput")

    xT3 = xT_d.ap().rearrange("(kc p) s -> p kc s", p=P)     # [128, 8, 2048]
    wq3 = wqT_d.ap().rearrange("(kc p) j -> p kc j", p=P)    # [128, 8, 512]
    wk3 = wkT_d.ap().rearrange("(kc p) j -> p kc j", p=P)
    wv3 = wvT_d.ap().rearrange("(kc p) j -> p kc j", p=P)
    wo3 = woT_d.ap().rearrange("(jc p) i -> p jc i", p=P)    # [128, 4, 1024]
    y_ap = y_d.ap()

    with tile.TileContext(nc) as tc:
        with (
            tc.tile_pool(name="pers", bufs=1) as pers,
            tc.tile_pool(name="w1", bufs=1) as w1,
            tc.tile_pool(name="x1", bufs=4) as x1,
            tc.tile_pool(name="sq", bufs=4) as sq,
            tc.tile_pool(name="tmp1", bufs=2) as tmp1,
            tc.tile_pool(name="ptp", bufs=3) as ptp,
            tc.tile_pool(name="rcp", bufs=4) as rcp,
            tc.tile_pool(name="rcd", bufs=2) as rcd,
            tc.tile_pool(name="rbp", bufs=3) as rbp,
            tc.tile_pool(name="wo", bufs=1) as wo,
            tc.tile_pool(name="ysb", bufs=2) as ysb,
            tc.tile_pool(name="drm", bufs=2, space="DRAM") as drm,
        ):
            qT = pers.tile([P, 4, S], F16)
            kT = pers.tile([P, 4, S], F16)
            vA = pers.tile([P, 16, 8, 65], F16)
            outT = pers.tile([P, 4, S], F16)
            cosb = pers.tile([P, S], F16)
            sinb = pers.tile([P, S], F16)
            trib = pers.tile([P, P], F16)
            rswp = pers.tile([P, P], F16)
            wq_s = w1.tile([P, 8, 512], F16)
            wk_s = w1.tile([P, 8, 512], F16)
            wv_s = w1.tile([P, 8, 512], F16)
            wo_s = wo.tile([P, 4, D], F16)
            xs_tiles = []
            for _sl in range(4):
                xs_t = x1.tile([P, 8, 512], F16, tag="xs")
                xs_tiles.append(xs_t)

            # DMA order: x slice 0 + q/k weights first (unblock the first
            # matmuls), batched as [P,4,512] transfers to cut dispatch count.
            for kh in range(2):
                k4 = slice(4 * kh, 4 * kh + 4)
                nc.sync.dma_start(xs_tiles[0][:, k4, :], xT3[:, k4, ts(0, 512)])
                nc.sync.dma_start(wq_s[:, k4, :], wq3[:, k4, :])
                nc.sync.dma_start(wk_s[:, k4, :], wk3[:, k4, :])
            nc.sync.dma_start(cosb[:], cosT_d.ap())
            nc.sync.dma_start(sinb[:], sinT_d.ap())
            nc.sync.dma_start(trib[:], tri_d.ap())
            nc.sync.dma_start(rswp[:], rswp_d.ap())
            for kh in range(2):
                k4 = slice(4 * kh, 4 * kh + 4)
                nc.sync.dma_start(xs_tiles[1][:, k4, :], xT3[:, k4, ts(1, 512)])
                nc.sync.dma_start(wv_s[:, k4, :], wv3[:, k4, :])
            for sl in (2, 3):
                for kh in range(2):
                    k4 = slice(4 * kh, 4 * kh + 4)
                    nc.sync.dma_start(
                        xs_tiles[sl][:, k4, :], xT3[:, k4, ts(sl, 512)]
                    )
            nc.sync.dma_start(wo_s[:], wo3)
            nc.vector.memset(vA[:, :, :, 64:65], 1.0)

            # PSUM: psP (proj/v/swap) 2 banks + psB (sc) 4 + psC (pa) 2 = 8
            _psP_cm = tc.tile_pool(name="psP", bufs=2, space="PSUM")
            _psB_cm = tc.tile_pool(name="psB", bufs=2, space="PSUM")
            _psC_cm = tc.tile_pool(name="psC", bufs=2, space="PSUM")
            psP = _psP_cm.__enter__()
            psB = _psB_cm.__enter__()
            psC = _psC_cm.__enter__()

            def rope(pq, dst2d, sls):
                pq_s = sq.tile([P, 512], F16, tag="pqs")
                nc.vector.tensor_copy(pq_s[:], pq[:])
                tA = tmp1.tile([P, 512], F16, tag="tA")
                nc.vector.tensor_mul(tA[:], pq_s[:], cosb[:, sls])
                qs = psP.tile([P, 512], F32, tag="pp")
                nc.tensor.matmul(qs[:], rswp[:], pq_s[:], start=True, stop=True)
                tBs = tmp1.tile([P, 512], F16, tag="tBs")
                nc.vector.tensor_mul(tBs[:], qs[:], sinb[:, sls])
                nc.vector.tensor_add(dst2d, tA[:], tBs[:])

            def proj_mms(hc, sl):
                # matmul halves of the q/k projections only -- the ropes are
                # deferred so they don't delay the enclosing attention
                # block's PSUM releases
                sls = ts(sl, 512)
                xs = xs_tiles[sl]
                prs = []
                for w_s, dstT in ((wq_s, qT), (wk_s, kT)):
                    pq = psP.tile([P, 512], F32, tag="pp")
                    for kc in range(8):
                        nc.tensor.matmul(
                            pq[:], w_s[:, kc, ts(hc, P)], xs[:, kc, :],
                            start=(kc == 0), stop=(kc == 7),
                        )
                    prs.append((pq, dstT[:, hc, sls], sls))
                return prs

            def do_ropes(prs):
                for pq, dst, sls in prs:
                    rope(pq, dst, sls)

            def proj_qk(hc, sl):
                do_ropes(proj_mms(hc, sl))

            def v_slice(sl):
                xs = xs_tiles[sl]
                for t4 in range(4):
                    pv = psP.tile([P, 512], F32, tag="pp")
                    for kc in range(8):
                        nc.tensor.matmul(
                            pv[:], xs[:, kc, ts(t4, P)], wv_s[:, kc, :],
                            start=(kc == 0), stop=(kc == 7),
                        )
                    nc.vector.tensor_copy(
                        vA[:, sl * 4 + t4, :, 0:64],
                        pv.rearrange("p (h c) -> p h c", h=8),
                    )

            def sc_mms(hc, j, i):
                # packed causal layout: head0 cols [w0:512] (q -> col q),
                # head1 cols [512:1024-w0] (q -> col 512+q-w0)
                w0 = max(i - 4 * j, 0) * P
                sc = psB.tile([P, 1024], F32, tag="sc")
                nc.tensor.matmul(
                    sc[:, w0:512], kT[0:64, hc, ts(i, P)],
                    qT[0:64, hc, j * 512 + w0 : (j + 1) * 512],
                    start=True, stop=True,
                )
                nc.tensor.matmul(
                    sc[:, 512 : 1024 - w0], kT[64:P, hc, ts(i, P)],
                    qT[64:P, hc, j * 512 + w0 : (j + 1) * 512],
                    start=True, stop=True,
                )
                return sc, w0

            def exp_pa(hc, j, i, sc, w0, pa0, pa1, last):
                # one contiguous exp over both heads' causal region; the
                # diagonal 128-blocks are zeroed post-exp by fp16 tri-mask
                # multiplies (the masked region of sc holds stale-but-finite
                # PSUM values, so exp is safe).
                diag = i - 4 * j >= 0
                pt = ptp.tile([P, 1024], F16, tag="pt")
                nc.scalar.activation(
                    pt[:, 0 : 1024 - 2 * w0], sc[:, w0 : 1024 - w0], Act.Exp
                )
                if diag:
                    nc.vector.tensor_mul(pt[:, 0:P], pt[:, 0:P], trib[:])
                    nc.vector.tensor_mul(
                        pt[:, 512 - w0 : 640 - w0],
                        pt[:, 512 - w0 : 640 - w0], trib[:],
                    )
                nc.tensor.matmul(
                    pa0[:, w0:512], vA[:, i, 2 * hc, :],
                    pt[:, 0 : 512 - w0],
                    start=(i == 0), stop=(i == last),
                )
                nc.tensor.matmul(
                    pa1[:, w0:512], vA[:, i, 2 * hc + 1, :],
                    pt[:, 512 - w0 : 1024 - 2 * w0],
                    start=(i == 0), stop=(i == last),
                )

            den_tiles = {}

            def attention_block(hc, j, den_d, rbase, filler=None):
                # filler: emits interleaved PE work (next pair's projection
                # matmuls) near the end of the block, so ScalarE's exp
                # backlog drains before the final AV matmuls need it
                pa0 = psC.tile([65, 512], F32, tag="pa")
                pa1 = psC.tile([65, 512], F32, tag="pa")
                last = 4 * j + 3
                pending = None
                sc_prev = sc_mms(hc, j, 0)
                for i in range(last + 1):
                    sc_next = sc_mms(hc, j, i + 1) if i < last else None
                    if i == last - 1 and filler is not None:
                        pending = filler()
                    exp_pa(hc, j, i, *sc_prev, pa0, pa1, last)
                    sc_prev = sc_next
                srows = []
                for h01, pa in ((0, pa0), (1, pa1)):
                    srow = rcp.tile([1, 512], F32, tag="srow")
                    nc.vector.tensor_copy(srow[:], pa[64:65, 0:512])
                    if den_d is not None:
                        nc.sync.dma_start(
                            den_d[rbase + h01 : rbase + h01 + 1, :], srow[:]
                        )
                    srows.append(srow)
                    nc.vector.tensor_copy(
                        outT[h01 * 64 : h01 * 64 + 64, hc, ts(j, 512)],
                        pa[0:64, 0:512],
                    )
                if pending is not None:
                    do_ropes(pending)
                return srows

            def epilogue_pair(hc, p3=None):
                den_sb = rcd.tile([8, 512], F32, tag="densb")
                nc.sync.dma_start(den_sb[:], den_tiles[hc][:])
                rec32 = rcd.tile([8, 512], F32, tag="rec32")
                nc.vector.reciprocal_approx_fast(rec32[:], den_sb[:])
                rec8 = rcd.tile([8, 512], F16, tag="rec8")
                with nc.allow_low_precision(reason="fp16 softmax denom"):
                    nc.vector.tensor_copy(rec8[:], rec32[:])
                rec_d = drm.tile([8, 512], F16, tag="recd")
                nc.sync.dma_start(rec_d[:], rec8[:])
                for j in range(4):
                    rb = rbp.tile([P, 512], F16, tag="rb")
                    for h01 in range(2):
                        r = 2 * j + h01
                        nc.sync.dma_start(
                            rb[h01 * 64 : h01 * 64 + 64, :],
                            rec_d[r : r + 1, :].broadcast_to((64, 512)),
                        )
                    nc.vector.tensor_mul(
                        outT[:, hc, ts(j, 512)], outT[:, hc, ts(j, 512)], rb[:]
                    )
                    if p3 is not None:
                        p3(j)

            # ---- output projection helper: runs during pair 3's attention,
            # borrowing psP's two banks (no projections remain there) ----
            def p3_group(j):
                for st in range(4 * j, 4 * j + 4):
                    py0 = psP.tile([P, 512], F32, tag="pp")
                    py1 = psP.tile([P, 512], F32, tag="pp")
                    # jc-outer so the two halves share each outT stationary
                    for jc in range(4):
                        nc.tensor.matmul(
                            py0[:], outT[:, jc, ts(st, P)], wo_s[:, jc, 0:512],
                            start=(jc == 0), stop=(jc == 3),
                        )
                        nc.tensor.matmul(
                            py1[:], outT[:, jc, ts(st, P)], wo_s[:, jc, 512:D],
                            start=(jc == 0), stop=(jc == 3),
                        )
                    yo0 = ysb.tile([P, 512], F32, tag="yo0")
                    yo1 = ysb.tile([P, 512], F32, tag="yo1")
                    nc.vector.tensor_copy(yo0[:], py0[:])
                    nc.vector.tensor_copy(yo1[:], py1[:])
                    nc.sync.dma_start(y_ap[ts(st, P), 0:512], yo0[:])
                    nc.sync.dma_start(y_ap[ts(st, P), 512:D], yo1[:])

            def epilogue_j3(j, srows):
                # per-j denominator chain for the last pair: direct
                # reciprocal on the partition-0 srow tiles, then a PE
                # ones-matmul broadcast into PSUM (trib row 0 is all-ones;
                # no DRAM hops on this tail-critical path).
                rbp2 = psB.tile([P, 1024], F32, tag="sc")
                for h01 in range(2):
                    r32 = rcd.tile([1, 512], F32, tag="r32b")
                    nc.vector.reciprocal_approx_fast(r32[:], srows[h01][:])
                    r16 = rcd.tile([1, 512], F16, tag="r16b")
                    with nc.allow_low_precision(reason="fp16 softmax denom"):
                        nc.vector.tensor_copy(r16[:], r32[:])
                    nc.tensor.matmul(
                        rbp2[h01 * 64 : h01 * 64 + 64, 0:512],
                        trib[0:1, 0:64], r16[:],
                        start=True, stop=True,
                    )
                nc.vector.tensor_mul(
                    outT[:, 3, ts(j, 512)], outT[:, 3, ts(j, 512)],
                    rbp2[:, 0:512],
                )

            # ---- woven schedule ----
            # prologue: q/k for pair 0
            for sl in range(4):
                proj_qk(0, sl)
            for hc in range(3):
                den_d = drm.tile([8, 512], F32, tag="dend")
                den_tiles[hc] = den_d
                for j in range(4):
                    if hc == 0:
                        v_slice(j)
                    attention_block(
                        hc, j, den_d, 2 * j,
                        filler=lambda hcn=hc + 1, sl=j: proj_mms(hcn, sl),
                    )
                if hc >= 1:
                    epilogue_pair(hc - 1)
            # pair 3: per-j epilogues; p3_group(j) issues after the NEXT
            # attention block so its denominator chain hides under it
            for j in range(4):
                srows = attention_block(3, j, None, 0)
                if j == 0:
                    epilogue_pair(2)
                epilogue_j3(j, srows)
                if j >= 1:
                    p3_group(j - 1)
            p3_group(3)

            _psC_cm.__exit__(None, None, None)
            _psB_cm.__exit__(None, None, None)
            _psP_cm.__exit__(None, None, None)

    nc.compile()
    return nc


def prep_core_inputs(x, token_ids, Wq, Wk, Wv, Wo, core):
    b, half = divmod(core, 2)
    rows = []
    for h in range(half * 8, half * 8 + 8):
        base = h * DH
        rows.extend(base + np.arange(0, DH, 2))
        rows.extend(base + np.arange(1, DH, 2))
    rows = np.asarray(rows)
    cols = np.arange(half * 512, half * 512 + 512)

    f16 = np.float16
    f32 = np.float32
    inv = THETA ** (-np.arange(0, DH, 2, dtype=np.float64) / DH)
    ang = np.asarray(token_ids, dtype=np.float64)[None, :] * inv[:, None]
    cosT = np.tile(np.cos(ang), (4, 1)).astype(f16)
    # signed sin table, pre-swapped: the PE swap-matmul moves pq[swap(m)] to
    # row m, so row m carries the sign/sin for the ROPE pair of m
    # (rows 0:32 = -sin for the even-dim halves, 32:64 = +sin, tiled).
    sin_block = np.concatenate([-np.sin(ang), np.sin(ang)], axis=0)
    sinT = np.tile(sin_block, (2, 1)).astype(f16)
    tri = (np.arange(P)[:, None] <= np.arange(P)[None, :]).astype(f16)
    swap_idx = np.arange(P)
    swap_idx = np.where(swap_idx % 64 < 32, swap_idx + 32, swap_idx - 32)
    rswp = np.zeros((P, P), dtype=f16)
    rswp[swap_idx, np.arange(P)] = 1.0
    return {
        "xT": np.ascontiguousarray(np.asarray(x[b], f32).T.astype(f16)),
        "wqT": np.ascontiguousarray((np.asarray(Wq, f32)[rows] * 0.125).T.astype(f16)),
        "wkT": np.ascontiguousarray(np.asarray(Wk, f32)[rows].T.astype(f16)),
        "wvT": np.ascontiguousarray(np.asarray(Wv, f32)[cols].T.astype(f16)),
        "woT": np.ascontiguousarray(np.asarray(Wo, f32)[:, cols].T.astype(f16)),
        "cosT": cosT,
        "sinT": sinT,
        "tri": tri,
        "rswp": rswp,
    }


def get_nc():
    if "nc" not in _CACHE:
        _CACHE["nc"] = build_nc()
    return _CACHE["nc"]


def run_cores(in_maps, trace=False):
    from concourse.bass_utils import run_bass_kernel_spmd

    return run_bass_kernel_spmd(
        get_nc(), in_maps, core_ids=list(range(N_CORES)), trace=trace
    )


def combine(res):
    y = np.empty((B, S, D), np.float32)
    for b in range(B):
        y[b] = res.results[2 * b]["y"] + res.results[2 * b + 1]["y"]
    return y


def kernel(x, token_ids, Wq, Wk, Wv, Wo):
    in_maps = [
        prep_core_inputs(x, token_ids, Wq, Wk, Wv, Wo, c) for c in range(N_CORES)
    ]
    res = run_cores(in_maps)
    return combine(res)


# revision 48
# speedup vs baseline: 1.0589x; 1.0589x over previous
"""Causal multi-head self-attention (B=4, S=2048, D=1024, H=16) on 8 TRN2
NeuronCores.

Sharding: core c = (batch b = c//2, head-half = c%2). Each core computes, for
its batch and its 8 heads: QKV projections (+RoPE), causal softmax attention,
and a row-sharded output projection. The host sums the two partial y's per
batch.

v5 design:
  - fp16 everywhere on-chip except PSUM (fp32) and the y output; data ranges
    are small (|score| < 5, den < 4e3). fp16 gives 1 cyc/row matmuls at any
    moving width, 2x DVE tensor_tensor, and half the DMA/SBUF footprint.
  - Fully woven schedule: pair hc+1's q/k projections (and, on pair 0, the
    v projection slices) are interleaved between the attention j-blocks, so
    the PE never drains and the HAM clock stays at 2.4 GHz; ScalarE's exp
    stream (the 2nd-busiest engine) overlaps projection matmuls.
  - RoPE: DVE copies the projection PSUM to fp16 SBUF; the +-32 partition
    swap runs on the PE as a constant permutation matmul; DVE does one mul
    by the pre-swapped signed sin table and the final add (2x fp16 rate).
  - Scores are packed causally: head0 at cols [w0:512] and head1 at
    [512:1024-w0] of one PSUM pair-tile, so each step's exp is a single
    contiguous ScalarE instruction covering exactly the causal range.
  - Causal masking of the diagonal 128-block via two tiny fp16 tri-mask
    multiplies on DVE (2x rate); v carries a 65th ones-column so the AV
    matmul emits the softmax denominator as PSUM row 64.
  - Denominators: batched per pair -> DRAM bounce -> fast-approx reciprocal
    -> partition-broadcast DMA -> one fp16 multiply per (pair, j).
"""

import numpy as np

B, S, D = 4, 2048, 1024
NUM_HEADS = 16
THETA = 10000.0
DH = 64
N_CORES = 8
P = 128

_CACHE = {}


def build_nc():
    """Build the single-core SPMD Bass program (identical on all 8 cores)."""
    import concourse.mybir as mybir
    import concourse.tile as tile
    from concourse import bacc
    from concourse.bass import ts

    F16 = mybir.dt.float16
    F32 = mybir.dt.float32
    Act = mybir.ActivationFunctionType

    nc = bacc.Bacc(trn_type="TRN2")
    xT_d = nc.dram_tensor("xT", [D, S], F16, kind="ExternalInput")
    wqT_d = nc.dram_tensor("wqT", [D, 512], F16, kind="ExternalInput")
    wkT_d = nc.dram_tensor("wkT", [D, 512], F16, kind="ExternalInput")
    wvT_d = nc.dram_tensor("wvT", [D, 512], F16, kind="ExternalInput")
    woT_d = nc.dram_tensor("woT", [512, D], F16, kind="ExternalInput")
    cosT_d = nc.dram_tensor("cosT", [P, S], F16, kind="ExternalInput")
    sinT_d = nc.dram_tensor("sinT", [P, S], F16, kind="ExternalInput")
    tri_d = nc.dram_tensor("tri", [P, P], F16, kind="ExternalInput")
    rswp_d = nc.dram_tensor("rswp", [P, P], F16, kind="ExternalInput")
    y_d = nc.dram_tensor("y", [S, D], F32, kind="ExternalOutput")

    xT3 = xT_d.ap().rearrange("(kc p) s -> p kc s", p=P)     # [128, 8, 2048]
    wq3 = wqT_d.ap().rearrange("(kc p) j -> p kc j", p=P)    # [128, 8, 512]
    wk3 = wkT_d.ap().rearrange("(kc p) j -> p kc j", p=P)
    wv3 = wvT_d.ap().rearrange("(kc p) j -> p kc j", p=P)
    wo3 = woT_d.ap().rearrange("(jc p) i -> p jc i", p=P)    # [128, 4, 1024]
    y_ap = y_d.ap()

    with tile.TileContext(nc) as tc:
        with (
            tc.tile_pool(name="pers", bufs=1) as pers,
            tc.tile_pool(name="w1", bufs=1) as w1,
            tc.tile_pool(name="x1", bufs=4) as x1,
            tc.tile_pool(name="sq", bufs=4) as sq,
            tc.tile_pool(name="tmp1", bufs=2) as tmp1,
            tc.tile_pool(name="ptp", bufs=3) as ptp,
            tc.tile_pool(name="rcp", bufs=4) as rcp,
            tc.tile_pool(name="rcd", bufs=2) as rcd,
            tc.tile_pool(name="rbp", bufs=3) as rbp,
            tc.tile_pool(name="wo", bufs=1) as wo,
            tc.tile_pool(name="ysb", bufs=2) as ysb,
            tc.tile_pool(name="drm", bufs=2, space="DRAM") as drm,
        ):
            qT = pers.tile([P, 4, S], F16)
            kT = pers.tile([P, 4, S], F16)
            vA = pers.tile([P, 16, 8, 65], F16)
            outT = pers.tile([P, 4, S], F16)
            cosb = pers.tile([P, S], F16)
            sinb = pers.tile([P, S], F16)
            trib = pers.tile([P, P], F16)
            rswp = pers.tile([P, P], F16)
            wq_s = w1.tile([P, 8, 512], F16)
            wk_s = w1.tile([P, 8, 512], F16)
            wv_s = w1.tile([P, 8, 512], F16)
            wo_s = wo.tile([P, 4, D], F16)
            xs_tiles = []
            for _sl in range(4):
                xs_t = x1.tile([P, 8, 512], F16, tag="xs")
                xs_tiles.append(xs_t)

            # DMA order: x slice 0 + q/k weights first (unblock the first
            # matmuls), batched as [P,4,512] transfers to cut dispatch count.
            for kh in range(2):
                k4 = slice(4 * kh, 4 * kh + 4)
                nc.sync.dma_start(xs_tiles[0][:, k4, :], xT3[:, k4, ts(0, 512)])
                nc.sync.dma_start(wq_s[:, k4, :], wq3[:, k4, :])
                nc.sync.dma_start(wk_s[:, k4, :], wk3[:, k4, :])
            nc.sync.dma_start(cosb[:], cosT_d.ap())
            nc.sync.dma_start(sinb[:], sinT_d.ap())
            nc.sync.dma_start(trib[:], tri_d.ap())
            nc.sync.dma_start(rswp[:], rswp_d.ap())
            for kh in range(2):
                k4 = slice(4 * kh, 4 * kh + 4)
                nc.sync.dma_start(xs_tiles[1][:, k4, :], xT3[:, k4, ts(1, 512)])
                nc.sync.dma_start(wv_s[:, k4, :], wv3[:, k4, :])
            for sl in (2, 3):
                for kh in range(2):
                    k4 = slice(4 * kh, 4 * kh + 4)
                    nc.sync.dma_start(
                        xs_tiles[sl][:, k4, :], xT3[:, k4, ts(sl, 512)]
                    )
            nc.sync.dma_start(wo_s[:], wo3)
            nc.vector.memset(vA[:, :, :, 64:65], 1.0)

            # PSUM: psP (proj/v/swap) 2 banks + psB (sc) 4 + psC (pa) 2 = 8
            _psP_cm = tc.tile_pool(name="psP", bufs=2, space="PSUM")
            _psB_cm = tc.tile_pool(name="psB", bufs=2, space="PSUM")
            _psC_cm = tc.tile_pool(name="psC", bufs=2, space="PSUM")
            psP = _psP_cm.__enter__()
            psB = _psB_cm.__enter__()
            psC = _psC_cm.__enter__()

            def rope(pq, dst2d, sls):
                # all-DVE: fp16 SBUF ops run at 2x, so the 4 partition-swap
                # muls cost ~314ns each; no PE involvement
                pq_s = sq.tile([P, 512], F16, tag="pqs")
                nc.vector.tensor_copy(pq_s[:], pq[:])
                tA = tmp1.tile([P, 512], F16, tag="tA")
                nc.vector.tensor_mul(tA[:], pq_s[:], cosb[:, sls])
                tBs = tmp1.tile([P, 512], F16, tag="tBs")
                for hb in (0, 64):
                    nc.vector.tensor_mul(
                        tBs[hb : hb + 32, :],
                        pq_s[hb + 32 : hb + 64, :],
                        sinb[hb + 32 : hb + 64, sls],
                    )
                    nc.vector.tensor_mul(
                        tBs[hb + 32 : hb + 64, :],
                        pq_s[hb : hb + 32, :],
                        sinb[hb : hb + 32, sls],
                    )
                nc.vector.tensor_add(dst2d, tA[:], tBs[:])

            def proj_mms(hc, sl):
                # matmul halves of the q/k projections only -- the ropes are
                # deferred so they don't delay the enclosing attention
                # block's PSUM releases
                sls = ts(sl, 512)
                xs = xs_tiles[sl]
                prs = []
                for w_s, dstT in ((wq_s, qT), (wk_s, kT)):
                    pq = psP.tile([P, 512], F32, tag="pp")
                    for kc in range(8):
                        nc.tensor.matmul(
                            pq[:], w_s[:, kc, ts(hc, P)], xs[:, kc, :],
                            start=(kc == 0), stop=(kc == 7),
                        )
                    prs.append((pq, dstT[:, hc, sls], sls))
                return prs

            def do_ropes(prs):
                for pq, dst, sls in prs:
                    rope(pq, dst, sls)

            def proj_qk(hc, sl):
                do_ropes(proj_mms(hc, sl))

            def v_slice(sl):
                xs = xs_tiles[sl]
                for t4 in range(4):
                    pv = psP.tile([P, 512], F32, tag="pp")
                    for kc in range(8):
                        nc.tensor.matmul(
                            pv[:], xs[:, kc, ts(t4, P)], wv_s[:, kc, :],
                            start=(kc == 0), stop=(kc == 7),
                        )
                    nc.vector.tensor_copy(
                        vA[:, sl * 4 + t4, :, 0:64],
                        pv.rearrange("p (h c) -> p h c", h=8),
                    )

            def sc_mms(hc, j, i):
                # packed causal layout: head0 cols [w0:512] (q -> col q),
                # head1 cols [512:1024-w0] (q -> col 512+q-w0)
                w0 = max(i - 4 * j, 0) * P
                sc = psB.tile([P, 1024], F32, tag="sc")
                nc.tensor.matmul(
                    sc[:, w0:512], kT[0:64, hc, ts(i, P)],
                    qT[0:64, hc, j * 512 + w0 : (j + 1) * 512],
                    start=True, stop=True,
                )
                nc.tensor.matmul(
                    sc[:, 512 : 1024 - w0], kT[64:P, hc, ts(i, P)],
                    qT[64:P, hc, j * 512 + w0 : (j + 1) * 512],
                    start=True, stop=True,
                )
                return sc, w0

            def exp_pa(hc, j, i, sc, w0, pa0, pa1, last):
                # one contiguous exp over both heads' causal region; the
                # diagonal 128-blocks are zeroed post-exp by fp16 tri-mask
                # multiplies (the masked region of sc holds stale-but-finite
                # PSUM values, so exp is safe).
                diag = i - 4 * j >= 0
                pt = ptp.tile([P, 1024], F16, tag="pt")
                nc.scalar.activation(
                    pt[:, 0 : 1024 - 2 * w0], sc[:, w0 : 1024 - w0], Act.Exp
                )
                if diag:
                    nc.vector.tensor_mul(pt[:, 0:P], pt[:, 0:P], trib[:])
                    nc.vector.tensor_mul(
                        pt[:, 512 - w0 : 640 - w0],
                        pt[:, 512 - w0 : 640 - w0], trib[:],
                    )
                nc.tensor.matmul(
                    pa0[:, w0:512], vA[:, i, 2 * hc, :],
                    pt[:, 0 : 512 - w0],
                    start=(i == 0), stop=(i == last),
                )
                nc.tensor.matmul(
                    pa1[:, w0:512], vA[:, i, 2 * hc + 1, :],
                    pt[:, 512 - w0 : 1024 - 2 * w0],
                    start=(i == 0), stop=(i == last),
                )

            den_tiles = {}

            def attention_block(hc, j, den_d, rbase, filler=None):
                # filler: emits interleaved PE work (next pair's projection
                # matmuls) near the end of the block, so ScalarE's exp
                # backlog drains before the final AV matmuls need it
                pa0 = psC.tile([65, 512], F32, tag="pa")
                pa1 = psC.tile([65, 512], F32, tag="pa")
                last = 4 * j + 3
                pending = None
                sc_prev = sc_mms(hc, j, 0)
                for i in range(last + 1):
                    sc_next = sc_mms(hc, j, i + 1) if i < last else None
                    if i == last - 1 and filler is not None:
                        pending = filler()
                    exp_pa(hc, j, i, *sc_prev, pa0, pa1, last)
                    sc_prev = sc_next
                srows = []
                for h01, pa in ((0, pa0), (1, pa1)):
                    srow = rcp.tile([1, 512], F32, tag="srow")
                    nc.vector.tensor_copy(srow[:], pa[64:65, 0:512])
                    if den_d is not None:
                        nc.sync.dma_start(
                            den_d[rbase + h01 : rbase + h01 + 1, :], srow[:]
                        )
                    srows.append(srow)
                    nc.vector.tensor_copy(
                        outT[h01 * 64 : h01 * 64 + 64, hc, ts(j, 512)],
                        pa[0:64, 0:512],
                    )
                if pending is not None:
                    do_ropes(pending)
                return srows

            def epilogue_pair(hc, p3=None):
                den_sb = rcd.tile([8, 512], F32, tag="densb")
                nc.sync.dma_start(den_sb[:], den_tiles[hc][:])
                rec32 = rcd.tile([8, 512], F32, tag="rec32")
                nc.vector.reciprocal_approx_fast(rec32[:], den_sb[:])
                rec8 = rcd.tile([8, 512], F16, tag="rec8")
                with nc.allow_low_precision(reason="fp16 softmax denom"):
                    nc.vector.tensor_copy(rec8[:], rec32[:])
                rec_d = drm.tile([8, 512], F16, tag="recd")
                nc.sync.dma_start(rec_d[:], rec8[:])
                for j in range(4):
                    rb = rbp.tile([P, 512], F16, tag="rb")
                    for h01 in range(2):
                        r = 2 * j + h01
                        nc.sync.dma_start(
                            rb[h01 * 64 : h01 * 64 + 64, :],
                            rec_d[r : r + 1, :].broadcast_to((64, 512)),
                        )
                    nc.vector.tensor_mul(
                        outT[:, hc, ts(j, 512)], outT[:, hc, ts(j, 512)], rb[:]
                    )
                    if p3 is not None:
                        p3(j)

            # ---- output projection helper: runs during pair 3's attention,
            # borrowing psP's two banks (no projections remain there) ----
            def p3_group(j):
                for st in range(4 * j, 4 * j + 4):
                    py0 = psP.tile([P, 512], F32, tag="pp")
                    py1 = psP.tile([P, 512], F32, tag="pp")
                    # jc-outer so the two halves share each outT stationary
                    for jc in range(4):
                        nc.tensor.matmul(
                            py0[:], outT[:, jc, ts(st, P)], wo_s[:, jc, 0:512],
                            start=(jc == 0), stop=(jc == 3),
                        )
                        nc.tensor.matmul(
                            py1[:], outT[:, jc, ts(st, P)], wo_s[:, jc, 512:D],
                            start=(jc == 0), stop=(jc == 3),
                        )
                    yo0 = ysb.tile([P, 512], F32, tag="yo0")
                    yo1 = ysb.tile([P, 512], F32, tag="yo1")
                    nc.vector.tensor_copy(yo0[:], py0[:])
                    nc.vector.tensor_copy(yo1[:], py1[:])
                    nc.sync.dma_start(y_ap[ts(st, P), 0:512], yo0[:])
                    nc.sync.dma_start(y_ap[ts(st, P), 512:D], yo1[:])

            def epilogue_j3(j, srows):
                # per-j denominator chain for the last pair: direct
                # reciprocal on the partition-0 srow tiles (no batching
                # bounce), one DRAM hop for the partition-broadcast.
                rec_d2 = drm.tile([2, 512], F16, tag="recd2")
                for h01 in range(2):
                    r32 = rcd.tile([1, 512], F32, tag="r32b")
                    nc.vector.reciprocal_approx_fast(r32[:], srows[h01][:])
                    r16 = rcd.tile([1, 512], F16, tag="r16b")
                    with nc.allow_low_precision(reason="fp16 softmax denom"):
                        nc.vector.tensor_copy(r16[:], r32[:])
                    nc.sync.dma_start(rec_d2[h01 : h01 + 1, :], r16[:])
                rb = rbp.tile([P, 512], F16, tag="rb")
                for h01 in range(2):
                    nc.sync.dma_start(
                        rb[h01 * 64 : h01 * 64 + 64, :],
                        rec_d2[h01 : h01 + 1, :].broadcast_to((64, 512)),
                    )
                nc.vector.tensor_mul(
                    outT[:, 3, ts(j, 512)], outT[:, 3, ts(j, 512)], rb[:]
                )

            # ---- woven schedule ----
            # prologue: q/k for pair 0
            for sl in range(4):
                proj_qk(0, sl)
            for hc in range(3):
                den_d = drm.tile([8, 512], F32, tag="dend")
                den_tiles[hc] = den_d
                for j in range(4):
                    if hc == 0:
                        v_slice(j)
                    attention_block(
                        hc, j, den_d, 2 * j,
                        filler=lambda hcn=hc + 1, sl=j: proj_mms(hcn, sl),
                    )
                if hc >= 1:
                    epilogue_pair(hc - 1)
            # pair 3: per-j epilogues; p3_group(j) issues after the NEXT
            # attention block so its denominator chain hides under it
            for j in range(4):
                srows = attention_block(3, j, None, 0)
                if j == 0:
                    epilogue_pair(2)
                epilogue_j3(j, srows)
                if j >= 1:
                    p3_group(j - 1)
            p3_group(3)

            _psC_cm.__exit__(None, None, None)
            _psB_cm.__exit__(None, None, None)
            _psP_cm.__exit__(None, None, None)

    nc.compile()
    return nc


def prep_core_inputs(x, token_ids, Wq, Wk, Wv, Wo, core):
    b, half = divmod(core, 2)
    rows = []
    for h in range(half * 8, half * 8 + 8):
        base = h * DH
        rows.extend(base + np.arange(0, DH, 2))
        rows.extend(base + np.arange(1, DH, 2))
    rows = np.asarray(rows)
    cols = np.arange(half * 512, half * 512 + 512)

    f16 = np.float16
    f32 = np.float32
    inv = THETA ** (-np.arange(0, DH, 2, dtype=np.float64) / DH)
    ang = np.asarray(token_ids, dtype=np.float64)[None, :] * inv[:, None]
    cosT = np.tile(np.cos(ang), (4, 1)).astype(f16)
    # signed sin table, source-indexed: the swap-muls read pq_s and sinT at
    # the SOURCE partitions (rows 0:32 = +sin, 32:64 = -sin, tiled)
    sin_block = np.concatenate([np.sin(ang), -np.sin(ang)], axis=0)
    sinT = np.tile(sin_block, (2, 1)).astype(f16)
    tri = (np.arange(P)[:, None] <= np.arange(P)[None, :]).astype(f16)
    swap_idx = np.arange(P)
    swap_idx = np.where(swap_idx % 64 < 32, swap_idx + 32, swap_idx - 32)
    rswp = np.zeros((P, P), dtype=f16)
    rswp[swap_idx, np.arange(P)] = 1.0
    return {
        "xT": np.ascontiguousarray(np.asarray(x[b], f32).T.astype(f16)),
        "wqT": np.ascontiguousarray((np.asarray(Wq, f32)[rows] * 0.125).T.astype(f16)),
        "wkT": np.ascontiguousarray(np.asarray(Wk, f32)[rows].T.astype(f16)),
        "wvT": np.ascontiguousarray(np.asarray(Wv, f32)[cols].T.astype(f16)),
        "woT": np.ascontiguousarray(np.asarray(Wo, f32)[:, cols].T.astype(f16)),
        "cosT": cosT,
        "sinT": sinT,
        "tri": tri,
        "rswp": rswp,
    }


def get_nc():
    if "nc" not in _CACHE:
        _CACHE["nc"] = build_nc()
    return _CACHE["nc"]


def run_cores(in_maps, trace=False):
    from concourse.bass_utils import run_bass_kernel_spmd

    return run_bass_kernel_spmd(
        get_nc(), in_maps, core_ids=list(range(N_CORES)), trace=trace
    )


def combine(res):
    y = np.empty((B, S, D), np.float32)
    for b in range(B):
        y[b] = res.results[2 * b]["y"] + res.results[2 * b + 1]["y"]
    return y


def kernel(x, token_ids, Wq, Wk, Wv, Wo):
    in_maps = [
        prep_core_inputs(x, token_ids, Wq, Wk, Wv, Wo, c) for c in range(N_CORES)
    ]
    res = run_cores(in_maps)
    return combine(res)
